# revision 25
# baseline (speedup 1.0000x reference)
"""Location-dependent 3D conv (AsymConv) on 8 TRN2 NeuronCores.

Math (per output voxel):
    out[b, 0, x, y, z] = sum_{i,j,l in 0..2} Xp[b, x+i, y+j, z+l] * W[x, y, z, (i*3+j)*3+l]
with Xp = edge-padded X by 1 plane on each spatial side.

Strategy (128-partition z-split layout):
  - Shard the x axis across cores (12 planes each, halo 14) -> no inter-core
    communication.
  - Per core, partition rows r = (zq, y) with zq a z-quarter and y the full
    96: 384 rows = exactly 3 "slots" of 128 partitions, so every DVE op uses
    all 128 lanes (a plain y-partition layout only reaches 96/128). Batch b
    lives in the free dims and W (which has no batch dim) is read through a
    stride-0 broadcast AP, so each W element moves from HBM exactly once:
    per-core traffic 5.97 MB W + 0.84 MB X + 0.44 MB out ~= 7.3 MB.
  - The host ships a per-partition halo slab [b, y' 3, x 14, zw 26]; all 27
    taps become free-dim offsets (j, i, l) into it - no partition-dim shifts
    anywhere, no halo exchange.
  - Products run on the Vector engine as fp16 tensor_tensor (2x_1p mode,
    ~2 elem/cycle/lane; scalar_tensor_tensor's nominal 4x mode measures 1x
    on HW, and GpSimd multiplies stall concurrent DVE ops ~4x via SBUF-port
    contention, so everything stays on DVE). walrus caps every engine AP at
    3 free dims, which forbids tap-pair fusion here: one op per tap,
    [slot*b (auto-fused), x, z]. l == 1 reads are 2B-misaligned (would drop
    2x), so those taps read a z-shifted slab copy built on the ScalarE
    mid-stream.
  - 27-term accumulation on the TensorEngine: identity[128x128] matmuls into
    4 PSUM chunks of 432 fp32 columns (512-col moving limit / 2KB banks).
    NOTE: matmul start=True zeroes the chunk's whole PSUM bank, not just the
    written window - only the first write per chunk may set it.
  - Head: the HWDGE completion-semaphore ladder retires ~10ns/descriptor
    serially per ring, so input DMAs are few, ordered by first use
    (w taps 0-1, slab slot 0, w taps 2-3, slots 1-2, then 4-tap W groups),
    and the first NSPLIT taps are computed as per-slot ops so DVE starts on
    slot 0's semaphore instead of slot 2's.
  - Tail: the final tap is computed per-slot with partial-window matmuls and
    each PSUM chunk is drained (DVE/ScalarE -> fp16 -> DMA out on two rings)
    as soon as its last contribution lands.
~48 us on HW vs 62.3 us for the previous y-partition kernel.
"""

import os

import numpy as np

# ---- problem constants (hardcoded per harness rules) ----
B = 2
D = 96  # Dx = Dy = Dz
KSZ = 3
NTAP = KSZ**3  # 27
NCORES = 8
XS = D // NCORES  # 12 x-planes per core
XH = XS + 2  # with halo
NSLOT = 3  # partition tile slots: 384 rows / 128
ZQ = 4  # z quarters
ZW = D // ZQ  # 24 output z per quarter
ZWH = ZW + 2  # z window incl halo
NCOL = NSLOT * B * XS * ZW  # 1728 psum columns
PCH = 4  # psum chunks
CCH = NCOL // PCH  # 432 columns per chunk

F16 = np.float16
LAST_RESULT = None  # BassKernelResults of the most recent run (for test.py)

_GRAPH_CACHE = {}

N_WARMUP = int(os.environ.get("ASYM_WARMUP", "0"))

# taps with l != 1 are 4B-aligned in the base slab; issue them first so the
# ScalarE z-shift copies (needed by l == 1 taps) are off the critical path
TAP_ORDER = [
    t
    for lgroup in (False, True)
    for j in range(KSZ)
    for t in range(NTAP)
    if (t % 3 == 1) == lgroup and (t // 3) % 3 == j
]
# W DMA groups (consumption order), 4 taps per transfer (2304B descriptors)
WG_BOUNDS = [0, 3, 6, 10, 14, 18, 22, 27]


def _build_graph():
    """Build (and cache) the per-core Bass graph. Same graph for all 8 cores."""
    key = ("nc", N_WARMUP)
    if key in _GRAPH_CACHE:
        return _GRAPH_CACHE[key]

    from concourse import bacc
    from concourse import bass as _bass
    import concourse.mybir as mybir
    from concourse.tile import TileContext

    f16 = mybir.dt.float16
    f32 = mybir.dt.float32
    MUL = mybir.AluOpType.mult

    nc = bacc.Bacc("TRN2", target_bir_lowering=False, debug=False, num_devices=NCORES)

    xs_d = nc.dram_tensor(
        "xslab", [128, NSLOT, B, KSZ, XH, ZWH], f16, kind="ExternalInput"
    )
    w_d = nc.dram_tensor("w", [128, NTAP, NSLOT, XS, ZW], f16, kind="ExternalInput")
    id_d = nc.dram_tensor("ident", [128, 128], f16, kind="ExternalInput")
    out_d = nc.dram_tensor("out", [128, NCOL], f16, kind="ExternalOutput")

    with TileContext(nc) as tc:
        with (
            tc.tile_pool(name="xp", bufs=1) as xpool,
            tc.tile_pool(name="wp", bufs=1) as wpool,
            tc.tile_pool(name="pp", bufs=4) as ppool,
            tc.tile_pool(name="psp", bufs=1, space="PSUM") as pspool,
        ):
            xslab = xpool.tile(
                [128, NSLOT, B, KSZ, XH, ZWH], f16, name="xslab", tag="xslab"
            )
            w_tiles = []  # per group
            for g in range(len(WG_BOUNDS) - 1):
                t0, t1 = WG_BOUNDS[g], WG_BOUNDS[g + 1]
                wg = wpool.tile(
                    [128, t1 - t0, NSLOT, XS, ZW], f16, name=f"w_{g}", tag=f"w_{g}"
                )
                w_tiles.append(wg)

            # All input DMAs on the SP ring, ordered by first use. The HWDGE
            # completion-sem ladder retires ~10ns/descriptor serially, so the
            # slab moves as 3 per-slot transfers (128 descriptors each): slot
            # 0's semaphore fires ~2.6us before a merged slab DMA's would,
            # and the per-slot leading taps (NSPLIT) start on it.
            nc.sync.dma_start(out=w_tiles[0][:], in_=w_d.ap()[:, 0:3])
            nc.sync.dma_start(out=xslab[:, 0:1], in_=xs_d.ap()[:, 0:1])
            nc.sync.dma_start(out=w_tiles[1][:], in_=w_d.ap()[:, 3:6])
            nc.sync.dma_start(out=xslab[:, 1:2], in_=xs_d.ap()[:, 1:2])
            nc.sync.dma_start(out=xslab[:, 2:3], in_=xs_d.ap()[:, 2:3])
            id_t = xpool.tile([128, 128], f16, name="id_t", tag="id_t")
            nc.scalar.dma_start(out=id_t[:], in_=id_d.ap())
            for g in range(2, len(WG_BOUNDS) - 1):
                t0, t1 = WG_BOUNDS[g], WG_BOUNDS[g + 1]
                nc.sync.dma_start(out=w_tiles[g][:], in_=w_d.ap()[:, t0:t1])

            # ---- z-shifted slab for l == 1 taps (2B-aligned reads);
            # copies are EMITTED mid-stream (see tap loop): instructions
            # emitted earlier on other engines can delay the DVE stream start.
            # rows stay ZWH(26)-wide so the x stride is 52B (4B-aligned on
            # every row, keeping DVE 2x); only cols 0..24 carry shifted data
            xz = xpool.tile(
                [128, NSLOT, B, KSZ, XH, ZWH], f16, name="xz", tag="xz"
            )

            def emit_xz_copies():
                for s in range(NSLOT):
                    for b in range(B):
                        nc.scalar.copy(
                            out=xz[:, s, b, :, :, 0 : ZWH - 1],
                            in_=xslab[:, s, b, :, :, 1:ZWH],
                        )

            # ---- PSUM accumulators ----
            psums = [
                pspool.tile([128, CCH], f32, name=f"ps_{ci}", tag=f"ps_{ci}")
                for ci in range(PCH)
            ]

            if N_WARMUP:
                dummy = ppool.tile([128, CCH], f16, name="warm", tag="warm", bufs=1)
                nc.vector.memset(dummy[:], 0.0)
                ps_w = pspool.tile([128, CCH], f32, name="ps_warm", tag="ps_warm")
                for _ in range(N_WARMUP):
                    nc.tensor.matmul(ps_w[:], id_t[:], dummy[:], start=True, stop=True)

            # ---- product + accumulate stream ----
            # walrus limits every engine to 3 free AP dims, so each tap is
            # one op: in0 [slot*b (fused), x, z], in1 W b-broadcast, out dense
            acc_cnt = 0  # taps accumulated so far (stop flags)
            # start=True zeroes the ENTIRE psum bank (not just the written
            # window): only the first matmul touching each psum tile may set
            # it; later partial windows accumulate onto the bank's zeros.
            ps_started = [False] * PCH

            def mm_consume(prod):
                nonlocal acc_cnt
                pbase = prod[:]
                for ci in range(PCH):
                    rhs = _bass.AP(
                        pbase.tensor,
                        pbase.offset + ci * CCH,
                        [pbase.ap[0], [1, CCH]],
                    )
                    nc.tensor.matmul(
                        psums[ci][:],
                        id_t[:],
                        rhs,
                        start=not ps_started[ci],
                        stop=(acc_cnt == NTAP - 1),
                    )
                    ps_started[ci] = True
                acc_cnt += 1

            def wg_of(wi):
                for g in range(len(WG_BOUNDS) - 1):
                    if WG_BOUNDS[g] <= wi < WG_BOUNDS[g + 1]:
                        return w_tiles[g], wi - WG_BOUNDS[g]
                raise AssertionError(wi)

            SSPLIT = 576  # flat columns per slot
            NSPLIT = int(os.environ.get("ASYM_NSPLIT", "6"))  # leading taps computed per-slot

            def mm_consume_slot(prod, s, first, last):
                # accumulate a per-slot product [128, 576] into the chunks it
                # overlaps (psum cols are 432-wide, slots 576-wide)
                pbase = prod[:]
                for ci in range(PCH):
                    lo = max(SSPLIT * s, CCH * ci)
                    hi = min(SSPLIT * s + SSPLIT, CCH * ci + CCH)
                    if lo >= hi:
                        continue
                    rhs = _bass.AP(
                        pbase.tensor,
                        pbase.offset + lo - SSPLIT * s,
                        [pbase.ap[0], [1, hi - lo]],
                    )
                    nc.tensor.matmul(
                        psums[ci][:, lo - CCH * ci : hi - CCH * ci],
                        id_t[:],
                        rhs,
                        start=not ps_started[ci],
                        stop=last,
                    )
                    ps_started[ci] = True

            # leading NSPLIT taps: one op per (slot, tap), emitted slot-major
            # so DVE work begins as soon as slot 0 + its W group have landed
            for s in range(NSLOT):
                for wi in range(NSPLIT):
                    t = TAP_ORDER[wi]
                    i, j, l = t // 9, (t // 3) % 3, t % 3
                    assert l != 1
                    wg, kk = wg_of(wi)
                    in0 = xslab[:, s, :, j, i : i + XS, l : l + ZW]
                    in1 = (
                        wg[:, kk, s].unsqueeze(1).broadcast_to([128, B, XS, ZW])
                    )
                    prod = ppool.tile(
                        [128, B, XS, ZW], f16, name="prodh", tag="prodh",
                        bufs=12,
                    )
                    nc.vector.tensor_tensor(out=prod[:], in0=in0, in1=in1, op=MUL)
                    mm_consume_slot(prod, s, wi == 0, wi == NTAP - 1)
                    if s == 0:
                        acc_cnt += 1

            for wi in range(NSPLIT, NTAP - 1):
                if wi == 8:
                    emit_xz_copies()
                t = TAP_ORDER[wi]
                i, j, l = t // 9, (t // 3) % 3, t % 3
                wg, kk = wg_of(wi)
                if l == 1:
                    in0 = xz[:, :, :, j, i : i + XS, 0:ZW]
                else:
                    in0 = xslab[:, :, :, j, i : i + XS, l : l + ZW]
                in1 = (
                    wg[:, kk].unsqueeze(2).broadcast_to([128, NSLOT, B, XS, ZW])
                )
                prod = ppool.tile(
                    [128, NSLOT, B, XS, ZW], f16, name="prods", tag="prods",
                    bufs=6,
                )
                nc.vector.tensor_tensor(out=prod[:], in0=in0, in1=in1, op=MUL)
                mm_consume(prod)

            # ---- final tap per-slot, draining each PSUM chunk as soon as
            # its last contribution lands (pipelines drain + out DMA with the
            # remaining products instead of serializing after the last op) ----
            wi = NTAP - 1
            t = TAP_ORDER[wi]
            i, j, l = t // 9, (t // 3) % 3, t % 3
            assert l == 1
            wg, kk = wg_of(wi)

            outsb = [
                ppool.tile([128, CCH], f16, name="outsb", tag=f"outsb_{ci}", bufs=1)
                for ci in range(PCH)
            ]

            def drain(ci, eng):
                eng.tensor_copy(out=outsb[ci][:], in_=psums[ci][:]) if eng is nc.vector else eng.copy(out=outsb[ci][:], in_=psums[ci][:])
                oq = (nc.sync, nc.scalar, nc.sync, nc.scalar)[ci]
                oq.dma_start(
                    out=out_d.ap()[:, ci * CCH : (ci + 1) * CCH], in_=outsb[ci][:]
                )

            # per-slot windows: (slot, [(ci, stop)]), then chunks completed
            FIN = [
                (0, [(0, True), (1, False)], [(0, nc.scalar)]),
                (1, [(1, True), (2, False)], [(1, nc.vector)]),
                (2, [(2, True), (3, True)], [(2, nc.scalar), (3, nc.vector)]),
            ]
            for s, wins, drains in FIN:
                in0 = xz[:, s, :, j, i : i + XS, 0:ZW]
                in1 = wg[:, kk, s].unsqueeze(1).broadcast_to([128, B, XS, ZW])
                prod = ppool.tile(
                    [128, B, XS, ZW], f16, name="prodf", tag="prodf", bufs=3
                )
                nc.vector.tensor_tensor(out=prod[:], in0=in0, in1=in1, op=MUL)
                pbase = prod[:]
                for ci, stp in wins:
                    lo = max(SSPLIT * s, CCH * ci)
                    hi = min(SSPLIT * s + SSPLIT, CCH * ci + CCH)
                    rhs = _bass.AP(
                        pbase.tensor,
                        pbase.offset + lo - SSPLIT * s,
                        [pbase.ap[0], [1, hi - lo]],
                    )
                    nc.tensor.matmul(
                        psums[ci][:, lo - CCH * ci : hi - CCH * ci],
                        id_t[:],
                        rhs,
                        start=not ps_started[ci],
                        stop=stp,
                    )
                    ps_started[ci] = True
                for ci, eng in drains:
                    drain(ci, eng)
            acc_cnt += 1
            assert acc_cnt == NTAP

    nc.compile()
    _GRAPH_CACHE[key] = nc
    return nc


def make_in_maps(X, W):
    """Host-side shard prep. X [2,1,96,96,96] f32, W [1,1,96,96,96,27] f32."""
    from numpy.lib.stride_tricks import sliding_window_view

    X = np.asarray(X)
    W = np.asarray(W)
    Xs = X.reshape(B, D, D, D)  # [b, x, y, z]
    # edge padding on all three spatial dims
    Xp = np.pad(Xs, ((0, 0), (1, 1), (1, 1), (1, 1)), mode="edge").astype(F16)
    # windows over (y, z): [b, xp 98, y0 96, z0 73, y' 3, zz 26]
    swv = sliding_window_view(Xp, (KSZ, ZWH), axis=(2, 3))
    W00 = W.reshape(D, D, D, NTAP).astype(F16)  # [x, y, z, t]
    ident = np.eye(128, dtype=F16)
    tap_perm = np.array(TAP_ORDER)

    in_maps = []
    for m in range(NCORES):
        # slab[r=(zq*96+y), b, y', xi, zz] with xi the 14-wide core x window
        arr = swv[:, m * XS : m * XS + XH, :, 0 : 3 * ZW + 1 : ZW]
        # arr: [b, xi 14, y 96, zq 4, y' 3, zz 26] -> [zq, y, b, y', xi, zz]
        slab = np.ascontiguousarray(np.transpose(arr, (3, 2, 0, 4, 1, 5))).reshape(
            NSLOT, 128, B, KSZ, XH, ZWH
        )
        slab = np.ascontiguousarray(np.transpose(slab, (1, 0, 2, 3, 4, 5)))

        wc = W00[m * XS : (m + 1) * XS][..., tap_perm]  # [xo 12, y, z, t]
        wc = wc.reshape(XS, D, ZQ, ZW, NTAP)  # [xo, y, zq, zo, t]
        wc = np.ascontiguousarray(np.transpose(wc, (2, 1, 4, 0, 3))).reshape(
            NSLOT, 128, NTAP, XS, ZW
        )
        wc = np.ascontiguousarray(np.transpose(wc, (1, 2, 0, 3, 4)))

        in_maps.append({"xslab": slab, "w": wc, "ident": ident})
    return in_maps


def kernel(X, W):
    global LAST_RESULT
    from concourse.bass_utils import run_bass_kernel_spmd

    nc = _build_graph()
    in_maps = make_in_maps(X, W)
    trace = bool(int(os.environ.get("ASYM_TRACE", "0")))
    res = run_bass_kernel_spmd(
        nc, in_maps, core_ids=list(range(NCORES)), trace=trace
    )
    LAST_RESULT = res

    out = np.empty((B, 1, D, D, D), dtype=np.float32)
    for m in range(NCORES):
        r = res.results[m]["out"].astype(np.float32)  # [128, 1728]
        r = r.reshape(128, NSLOT, B, XS, ZW)
        r = np.transpose(r, (1, 0, 2, 3, 4)).reshape(ZQ, D, B, XS, ZW)
        # [zq, y, b, xo, zo] -> [b, xo, y, zq, zo]
        r = np.transpose(r, (2, 3, 1, 0, 4)).reshape(B, XS, D, D)
        out[:, 0, m * XS : (m + 1) * XS, :, :] = r
    return out


# revision 26
# speedup vs baseline: 1.1460x; 1.1460x over previous
"""Location-dependent 3D conv (AsymConv) on 8 TRN2 NeuronCores.

Math (per output voxel):
    out[b, 0, x, y, z] = sum_{i,j,l in 0..2} Xp[b, x+i, y+j, z+l] * W[x, y, z, (i*3+j)*3+l]
with Xp = edge-padded X by 1 plane on each spatial side.

Strategy (128-partition z-split layout):
  - Shard the x axis across cores (12 planes each, halo 14) -> no inter-core
    communication.
  - Per core, partition rows r = (zq, y) with zq a z-quarter and y the full
    96: 384 rows = exactly 3 "slots" of 128 partitions, so every DVE op uses
    all 128 lanes (a plain y-partition layout only reaches 96/128). Batch b
    lives in the free dims and W (which has no batch dim) is read through a
    stride-0 broadcast AP, so each W element moves from HBM exactly once:
    per-core traffic 5.97 MB W + 0.84 MB X + 0.44 MB out ~= 7.3 MB.
  - The host ships a per-partition halo slab [b, y' 3, x 14, zw 26]; all 27
    taps become free-dim offsets (j, i, l) into it - no partition-dim shifts
    anywhere, no halo exchange.
  - Products run on the Vector engine as fp16 tensor_tensor (2x_1p mode,
    ~2 elem/cycle/lane; scalar_tensor_tensor's nominal 4x mode measures 1x
    on HW, and GpSimd multiplies stall concurrent DVE ops ~4x via SBUF-port
    contention, so everything stays on DVE). walrus caps every engine AP at
    3 free dims, which forbids tap-pair fusion here: one op per tap,
    [slot*b (auto-fused), x, z]. l == 1 reads are 2B-misaligned (would drop
    2x), so those taps read a z-shifted slab copy built on the ScalarE
    mid-stream.
  - 27-term accumulation on the TensorEngine: identity[128x128] matmuls into
    4 PSUM chunks of 432 fp32 columns (512-col moving limit / 2KB banks).
    NOTE: matmul start=True zeroes the chunk's whole PSUM bank, not just the
    written window - only the first write per chunk may set it.
  - Head: the HWDGE completion-semaphore ladder retires ~10ns/descriptor
    serially per ring, so input DMAs are few, ordered by first use
    (w taps 0-1, slab slot 0, w taps 2-3, slots 1-2, then 4-tap W groups),
    and the first NSPLIT taps are computed as per-slot ops so DVE starts on
    slot 0's semaphore instead of slot 2's.
  - Tail: the final tap is computed per-slot with partial-window matmuls and
    each PSUM chunk is drained (DVE/ScalarE -> fp16 -> DMA out on two rings)
    as soon as its last contribution lands.
~48 us on HW vs 62.3 us for the previous y-partition kernel.
"""

import os

import numpy as np

# ---- problem constants (hardcoded per harness rules) ----
B = 2
D = 96  # Dx = Dy = Dz
KSZ = 3
NTAP = KSZ**3  # 27
NCORES = 8
XS = D // NCORES  # 12 x-planes per core
XH = XS + 2  # with halo
NSLOT = 3  # partition tile slots: 384 rows / 128
ZQ = 4  # z quarters
ZW = D // ZQ  # 24 output z per quarter
ZWH = ZW + 2  # z window incl halo
NCOL = NSLOT * B * XS * ZW  # 1728 psum columns
PCH = 4  # psum chunks
CCH = NCOL // PCH  # 432 columns per chunk

F16 = np.float16
LAST_RESULT = None  # BassKernelResults of the most recent run (for test.py)

_GRAPH_CACHE = {}

N_WARMUP = int(os.environ.get("ASYM_WARMUP", "0"))

# taps with l != 1 are 4B-aligned in the base slab; issue them first so the
# ScalarE z-shift copies (needed by l == 1 taps) are off the critical path
TAP_ORDER = [
    t
    for lgroup in (False, True)
    for j in range(KSZ)
    for t in range(NTAP)
    if (t % 3 == 1) == lgroup and (t // 3) % 3 == j
]
# W DMA groups (consumption order), 4 taps per transfer (2304B descriptors)
WG_BOUNDS = [0, 3, 6, 10, 14, 18, 22, 27]


def _build_graph():
    """Build (and cache) the per-core Bass graph. Same graph for all 8 cores."""
    key = ("nc", N_WARMUP)
    if key in _GRAPH_CACHE:
        return _GRAPH_CACHE[key]

    from concourse import bacc
    from concourse import bass as _bass
    import concourse.mybir as mybir
    from concourse.tile import TileContext

    f16 = mybir.dt.float16
    f32 = mybir.dt.float32
    MUL = mybir.AluOpType.mult

    nc = bacc.Bacc("TRN2", target_bir_lowering=False, debug=False, num_devices=NCORES)

    xs_d = nc.dram_tensor(
        "xslab", [128, NSLOT, B, KSZ, XH, ZWH], f16, kind="ExternalInput"
    )
    w_d = nc.dram_tensor("w", [128, NTAP, NSLOT, XS, ZW], f16, kind="ExternalInput")
    id_d = nc.dram_tensor("ident", [128, 128], f16, kind="ExternalInput")
    out_d = nc.dram_tensor("out", [128, NCOL], f16, kind="ExternalOutput")

    with TileContext(nc) as tc:
        with (
            tc.tile_pool(name="xp", bufs=1) as xpool,
            tc.tile_pool(name="wp", bufs=1) as wpool,
            tc.tile_pool(name="pp", bufs=4) as ppool,
            tc.tile_pool(name="psp", bufs=1, space="PSUM") as pspool,
        ):
            xslab = xpool.tile(
                [128, NSLOT, B, KSZ, XH, ZWH], f16, name="xslab", tag="xslab"
            )
            w_tiles = []  # per group
            for g in range(len(WG_BOUNDS) - 1):
                t0, t1 = WG_BOUNDS[g], WG_BOUNDS[g + 1]
                wg = wpool.tile(
                    [128, t1 - t0, NSLOT, XS, ZW], f16, name=f"w_{g}", tag=f"w_{g}"
                )
                w_tiles.append(wg)

            # All input DMAs on the SP ring, ordered by first use. The HWDGE
            # completion-sem ladder retires ~10ns/descriptor serially, so the
            # slab moves as 3 per-slot transfers (128 descriptors each): slot
            # 0's semaphore fires ~2.6us before a merged slab DMA's would,
            # and the per-slot leading taps (NSPLIT) start on it.
            nc.sync.dma_start(out=w_tiles[0][:], in_=w_d.ap()[:, 0:3])
            nc.sync.dma_start(out=xslab[:, 0:1], in_=xs_d.ap()[:, 0:1])
            nc.sync.dma_start(out=xslab[:, 1:2], in_=xs_d.ap()[:, 1:2])
            nc.sync.dma_start(out=w_tiles[1][:], in_=w_d.ap()[:, 3:6])
            nc.sync.dma_start(out=xslab[:, 2:3], in_=xs_d.ap()[:, 2:3])
            id_t = xpool.tile([128, 128], f16, name="id_t", tag="id_t")
            nc.scalar.dma_start(out=id_t[:], in_=id_d.ap())
            for g in range(2, len(WG_BOUNDS) - 1):
                t0, t1 = WG_BOUNDS[g], WG_BOUNDS[g + 1]
                nc.sync.dma_start(out=w_tiles[g][:], in_=w_d.ap()[:, t0:t1])

            # ---- z-shifted slab for l == 1 taps (2B-aligned reads);
            # copies are EMITTED mid-stream (see tap loop): instructions
            # emitted earlier on other engines can delay the DVE stream start.
            # rows stay ZWH(26)-wide so the x stride is 52B (4B-aligned on
            # every row, keeping DVE 2x); only cols 0..24 carry shifted data
            xz = xpool.tile(
                [128, NSLOT, B, KSZ, XH, ZWH], f16, name="xz", tag="xz"
            )

            def emit_xz_copies():
                for s in range(NSLOT):
                    for b in range(B):
                        nc.scalar.copy(
                            out=xz[:, s, b, :, :, 0 : ZWH - 1],
                            in_=xslab[:, s, b, :, :, 1:ZWH],
                        )

            # ---- PSUM accumulators ----
            psums = [
                pspool.tile([128, CCH], f32, name=f"ps_{ci}", tag=f"ps_{ci}")
                for ci in range(PCH)
            ]

            if N_WARMUP:
                dummy = ppool.tile([128, CCH], f16, name="warm", tag="warm", bufs=1)
                nc.vector.memset(dummy[:], 0.0)
                ps_w = pspool.tile([128, CCH], f32, name="ps_warm", tag="ps_warm")
                for _ in range(N_WARMUP):
                    nc.tensor.matmul(ps_w[:], id_t[:], dummy[:], start=True, stop=True)

            # ---- product + accumulate stream ----
            # walrus limits every engine to 3 free AP dims, so each tap is
            # one op: in0 [slot*b (fused), x, z], in1 W b-broadcast, out dense
            acc_cnt = 0  # taps accumulated so far (stop flags)
            # start=True zeroes the ENTIRE psum bank (not just the written
            # window): only the first matmul touching each psum tile may set
            # it; later partial windows accumulate onto the bank's zeros.
            ps_started = [False] * PCH

            def mm_consume(prod):
                nonlocal acc_cnt
                pbase = prod[:]
                for ci in range(PCH):
                    rhs = _bass.AP(
                        pbase.tensor,
                        pbase.offset + ci * CCH,
                        [pbase.ap[0], [1, CCH]],
                    )
                    nc.tensor.matmul(
                        psums[ci][:],
                        id_t[:],
                        rhs,
                        start=not ps_started[ci],
                        stop=(acc_cnt == NTAP - 1),
                    )
                    ps_started[ci] = True
                acc_cnt += 1

            def wg_of(wi):
                for g in range(len(WG_BOUNDS) - 1):
                    if WG_BOUNDS[g] <= wi < WG_BOUNDS[g + 1]:
                        return w_tiles[g], wi - WG_BOUNDS[g]
                raise AssertionError(wi)

            SSPLIT = 576  # flat columns per slot
            NSPLIT = int(os.environ.get("ASYM_NSPLIT", "6"))  # leading taps computed per-slot

            def mm_consume_slot(prod, s, first, last):
                # accumulate a per-slot product [128, 576] into the chunks it
                # overlaps (psum cols are 432-wide, slots 576-wide)
                pbase = prod[:]
                for ci in range(PCH):
                    lo = max(SSPLIT * s, CCH * ci)
                    hi = min(SSPLIT * s + SSPLIT, CCH * ci + CCH)
                    if lo >= hi:
                        continue
                    rhs = _bass.AP(
                        pbase.tensor,
                        pbase.offset + lo - SSPLIT * s,
                        [pbase.ap[0], [1, hi - lo]],
                    )
                    nc.tensor.matmul(
                        psums[ci][:, lo - CCH * ci : hi - CCH * ci],
                        id_t[:],
                        rhs,
                        start=not ps_started[ci],
                        stop=last,
                    )
                    ps_started[ci] = True

            # leading NSPLIT taps: one op per (slot, tap), emitted slot-major
            # so DVE work begins as soon as slot 0 + its W group have landed
            # piece order matches input-arrival order (slot0/1 sems land
            # before W group 1, slot2 last)
            half = min(NSPLIT, len([t for t in range(WG_BOUNDS[1])]))
            piece_order = (
                [(0, wi) for wi in range(half)]
                + [(1, wi) for wi in range(half)]
                + [(0, wi) for wi in range(half, NSPLIT)]
                + [(1, wi) for wi in range(half, NSPLIT)]
                + [(2, wi) for wi in range(NSPLIT)]
            )
            for s, wi in piece_order:
                if True:
                    t = TAP_ORDER[wi]
                    i, j, l = t // 9, (t // 3) % 3, t % 3
                    assert l != 1
                    wg, kk = wg_of(wi)
                    in0 = xslab[:, s, :, j, i : i + XS, l : l + ZW]
                    in1 = (
                        wg[:, kk, s].unsqueeze(1).broadcast_to([128, B, XS, ZW])
                    )
                    prod = ppool.tile(
                        [128, B, XS, ZW], f16, name="prodh", tag="prodh",
                        bufs=12,
                    )
                    nc.vector.tensor_tensor(out=prod[:], in0=in0, in1=in1, op=MUL)
                    mm_consume_slot(prod, s, wi == 0, wi == NTAP - 1)
                    if s == 0:
                        acc_cnt += 1  # counts each tap once

            for wi in range(NSPLIT, NTAP - 1):
                if wi == 8:
                    emit_xz_copies()
                t = TAP_ORDER[wi]
                i, j, l = t // 9, (t // 3) % 3, t % 3
                wg, kk = wg_of(wi)
                if l == 1:
                    in0 = xz[:, :, :, j, i : i + XS, 0:ZW]
                else:
                    in0 = xslab[:, :, :, j, i : i + XS, l : l + ZW]
                in1 = (
                    wg[:, kk].unsqueeze(2).broadcast_to([128, NSLOT, B, XS, ZW])
                )
                prod = ppool.tile(
                    [128, NSLOT, B, XS, ZW], f16, name="prods", tag="prods",
                    bufs=6,
                )
                nc.vector.tensor_tensor(out=prod[:], in0=in0, in1=in1, op=MUL)
                mm_consume(prod)

            # ---- final tap per-slot, draining each PSUM chunk as soon as
            # its last contribution lands (pipelines drain + out DMA with the
            # remaining products instead of serializing after the last op) ----
            wi = NTAP - 1
            t = TAP_ORDER[wi]
            i, j, l = t // 9, (t // 3) % 3, t % 3
            assert l == 1
            wg, kk = wg_of(wi)

            outsb = [
                ppool.tile([128, CCH], f16, name="outsb", tag=f"outsb_{ci}", bufs=1)
                for ci in range(PCH)
            ]

            def drain(ci, eng):
                eng.tensor_copy(out=outsb[ci][:], in_=psums[ci][:]) if eng is nc.vector else eng.copy(out=outsb[ci][:], in_=psums[ci][:])
                oq = (nc.sync, nc.scalar, nc.sync, nc.scalar)[ci]
                oq.dma_start(
                    out=out_d.ap()[:, ci * CCH : (ci + 1) * CCH], in_=outsb[ci][:]
                )

            # per-slot windows: (slot, [(ci, stop)]), then chunks completed
            FIN = [
                (0, [(0, True), (1, False)], [(0, nc.scalar)]),
                (1, [(1, True), (2, False)], [(1, nc.vector)]),
                (2, [(2, True), (3, True)], [(2, nc.scalar), (3, nc.vector)]),
            ]
            for s, wins, drains in FIN:
                in0 = xz[:, s, :, j, i : i + XS, 0:ZW]
                in1 = wg[:, kk, s].unsqueeze(1).broadcast_to([128, B, XS, ZW])
                prod = ppool.tile(
                    [128, B, XS, ZW], f16, name="prodf", tag="prodf", bufs=3
                )
                nc.vector.tensor_tensor(out=prod[:], in0=in0, in1=in1, op=MUL)
                pbase = prod[:]
                for ci, stp in wins:
                    lo = max(SSPLIT * s, CCH * ci)
                    hi = min(SSPLIT * s + SSPLIT, CCH * ci + CCH)
                    rhs = _bass.AP(
                        pbase.tensor,
                        pbase.offset + lo - SSPLIT * s,
                        [pbase.ap[0], [1, hi - lo]],
                    )
                    nc.tensor.matmul(
                        psums[ci][:, lo - CCH * ci : hi - CCH * ci],
                        id_t[:],
                        rhs,
                        start=not ps_started[ci],
                        stop=stp,
                    )
                    ps_started[ci] = True
                for ci, eng in drains:
                    drain(ci, eng)
            acc_cnt += 1
            assert acc_cnt == NTAP

    nc.compile()
    _GRAPH_CACHE[key] = nc
    return nc


def make_in_maps(X, W):
    """Host-side shard prep. X [2,1,96,96,96] f32, W [1,1,96,96,96,27] f32."""
    from numpy.lib.stride_tricks import sliding_window_view

    X = np.asarray(X)
    W = np.asarray(W)
    Xs = X.reshape(B, D, D, D)  # [b, x, y, z]
    # edge padding on all three spatial dims
    Xp = np.pad(Xs, ((0, 0), (1, 1), (1, 1), (1, 1)), mode="edge").astype(F16)
    # windows over (y, z): [b, xp 98, y0 96, z0 73, y' 3, zz 26]
    swv = sliding_window_view(Xp, (KSZ, ZWH), axis=(2, 3))
    W00 = W.reshape(D, D, D, NTAP).astype(F16)  # [x, y, z, t]
    ident = np.eye(128, dtype=F16)
    tap_perm = np.array(TAP_ORDER)

    in_maps = []
    for m in range(NCORES):
        # slab[r=(zq*96+y), b, y', xi, zz] with xi the 14-wide core x window
        arr = swv[:, m * XS : m * XS + XH, :, 0 : 3 * ZW + 1 : ZW]
        # arr: [b, xi 14, y 96, zq 4, y' 3, zz 26] -> [zq, y, b, y', xi, zz]
        slab = np.ascontiguousarray(np.transpose(arr, (3, 2, 0, 4, 1, 5))).reshape(
            NSLOT, 128, B, KSZ, XH, ZWH
        )
        slab = np.ascontiguousarray(np.transpose(slab, (1, 0, 2, 3, 4, 5)))

        wc = W00[m * XS : (m + 1) * XS][..., tap_perm]  # [xo 12, y, z, t]
        wc = wc.reshape(XS, D, ZQ, ZW, NTAP)  # [xo, y, zq, zo, t]
        wc = np.ascontiguousarray(np.transpose(wc, (2, 1, 4, 0, 3))).reshape(
            NSLOT, 128, NTAP, XS, ZW
        )
        wc = np.ascontiguousarray(np.transpose(wc, (1, 2, 0, 3, 4)))

        in_maps.append({"xslab": slab, "w": wc, "ident": ident})
    return in_maps


def kernel(X, W):
    global LAST_RESULT
    from concourse.bass_utils import run_bass_kernel_spmd

    nc = _build_graph()
    in_maps = make_in_maps(X, W)
    trace = bool(int(os.environ.get("ASYM_TRACE", "0")))
    res = run_bass_kernel_spmd(
        nc, in_maps, core_ids=list(range(NCORES)), trace=trace
    )
    LAST_RESULT = res

    out = np.empty((B, 1, D, D, D), dtype=np.float32)
    for m in range(NCORES):
        r = res.results[m]["out"].astype(np.float32)  # [128, 1728]
        r = r.reshape(128, NSLOT, B, XS, ZW)
        r = np.transpose(r, (1, 0, 2, 3, 4)).reshape(ZQ, D, B, XS, ZW)
        # [zq, y, b, xo, zo] -> [b, xo, y, zq, zo]
        r = np.transpose(r, (2, 3, 1, 0, 4)).reshape(B, XS, D, D)
        out[:, 0, m * XS : (m + 1) * XS, :, :] = r
    return out
